# revision 6
# baseline (speedup 1.0000x reference)
"""Conv1D-MHSA (sketched linear attention) Trainium2 kernel.

Math: the reference computes, per (batch b, head h):
    q = conv1d_K3(x_pad, q_w) + q_b ; k likewise ; v = conv1d_K1(x, v_w)
    phi_q = sqrt(R) * tanh((q^T g1_q) * (q^T g2_q) / sqrt(R))  (phi_k likewise)
    scores = phi_q @ phi_k^T                     [L, L]
    o = (scores / (rowsum(scores) + 1e-6)) @ v   [L, D]
    out = concat_h(o) @ proj_w^T + proj_b

There is no softmax, so `o` is linear in `scores` and the L x L matrix is
never needed:
    o = diag(1/(phi_q @ s_k + eps)) . phi_q @ (phi_k^T v),   s_k = colsum(phi_k)
Furthermore the sqrt(R) post-scales on phi_q/phi_k cancel between numerator
and denominator, leaving only eps -> eps/R.

Sharding: head-parallel over 8 cores (head h -> core h, both batches).  Each
core returns a partial projection [B, L, D]; host sums the 8 partials and
adds proj_b.  gamma/beta affine and conv biases are folded into weights on
the host.
"""

import numpy as np
from contextlib import ExitStack

import concourse.bacc as bacc
import concourse.bass as bass
import concourse.mybir as mybir
import concourse.tile as tile
from concourse.bass_utils import run_bass_kernel_spmd
from concourse.masks import make_identity

F32 = mybir.dt.float32
AF = mybir.ActivationFunctionType

B = 2          # batch
D = 128        # per-head dim (= partition size)
L = 2048       # sequence length
H = 8          # heads == cores
R = 128        # sketch dim
KS = 3         # conv kernel size
LP = L + KS - 1
NCH = L // 512   # 4 big chunks
NT = L // 128    # 16 tiles
SQRT_R = float(np.sqrt(R))
EPS = float(1e-6 / R)

_built_nc = None
last_results = None


def _build():
    nc = bacc.Bacc(None, target_bir_lowering=False)
    xp_d = nc.declare_dram_parameter("xp", [D, B, LP], F32, isOutput=False)
    qkw_d = nc.declare_dram_parameter("qkw", [D, 2, KS, D], F32, isOutput=False)
    qkb_d = nc.declare_dram_parameter("qkb", [D, 2], F32, isOutput=False)
    vw_d = nc.declare_dram_parameter("vw", [D, D], F32, isOutput=False)
    g_d = nc.declare_dram_parameter("g", [D, 4, R], F32, isOutput=False)
    pw_d = nc.declare_dram_parameter("pw", [D, D], F32, isOutput=False)
    out_d = nc.declare_dram_parameter("outp", [B, L, D], F32, isOutput=True)

    with ExitStack() as ctx:
        tc = ctx.enter_context(tile.TileContext(nc))
        consts = ctx.enter_context(tc.tile_pool(name="consts", bufs=1))
        perb = ctx.enter_context(tc.tile_pool(name="perb", bufs=2))
        work = ctx.enter_context(tc.tile_pool(name="work", bufs=3))
        small = ctx.enter_context(tc.tile_pool(name="small", bufs=4))
        outs = ctx.enter_context(tc.tile_pool(name="outs", bufs=4))
        # PSUM: 8 banks total.  psA: 512-wide tiles (3 banks),
        # psB: <=129-wide tiles (4 banks), psM: the M~ accumulator (1 bank).
        psA = ctx.enter_context(tc.tile_pool(name="psA", bufs=3, space="PSUM"))
        psB = ctx.enter_context(tc.tile_pool(name="psB", bufs=4, space="PSUM"))
        psM = ctx.enter_context(tc.tile_pool(name="psM", bufs=1, space="PSUM"))

        ident = consts.tile([128, 128], F32, tag="ident")
        make_identity(nc, ident)
        xp_s = consts.tile([D, B, LP], F32, tag="xp")
        for b in range(B):
            nc.sync.dma_start(out=xp_s[:, b, :], in_=xp_d[:, b, :])
        qkw_s = consts.tile([D, 2, KS, D], F32, tag="qkw")
        nc.sync.dma_start(out=qkw_s, in_=qkw_d[:])
        qkb_s = consts.tile([D, 2], F32, tag="qkb")
        nc.sync.dma_start(out=qkb_s, in_=qkb_d[:])
        vw_s = consts.tile([D, D], F32, tag="vw")
        nc.sync.dma_start(out=vw_s, in_=vw_d[:])
        g_s = consts.tile([D, 4, R], F32, tag="g")
        nc.sync.dma_start(out=g_s, in_=g_d[:])
        pw_s = consts.tile([D, D], F32, tag="pw")
        nc.sync.dma_start(out=pw_s, in_=pw_d[:])

        for b in range(B):
            # ---- causal conv1d for q and k: qk[d, l] (PSUM-accumulated taps)
            qk_sb = perb.tile([D, 2, L], F32, tag="qk")
            for p in range(2):
                for c in range(NCH):
                    ps = psA.tile([128, 512], F32, tag="psA")
                    for t in range(KS):
                        nc.tensor.matmul(
                            ps,
                            lhsT=qkw_s[:, p, t, :],
                            rhs=xp_s[:, b, c * 512 + t : c * 512 + t + 512],
                            start=(t == 0),
                            stop=(t == KS - 1),
                        )
                    # evacuate + per-channel bias
                    nc.scalar.add(qk_sb[:, p, c * 512 : (c + 1) * 512], ps,
                                  qkb_s[:, p : p + 1])

            # ---- phi_q in [r, l] layout (tanh((u1*u2)/sqrt_r), no post-scale)
            phiq = perb.tile([R, L], F32, tag="phiq")
            for c in range(NCH):
                u1 = psA.tile([128, 512], F32, tag="psA")
                u2 = psA.tile([128, 512], F32, tag="psA")
                rhs = qk_sb[:, 0, c * 512 : (c + 1) * 512]
                nc.tensor.matmul(u1, lhsT=g_s[:, 0, :], rhs=rhs, start=True, stop=True)
                nc.tensor.matmul(u2, lhsT=g_s[:, 1, :], rhs=rhs, start=True, stop=True)
                # DVE can read only one PSUM operand per op: stage u1 in SBUF
                u1s = work.tile([128, 512], F32, tag="u1s")
                nc.vector.tensor_copy(u1s, u1)
                prod = work.tile([128, 512], F32, tag="prod")
                nc.vector.tensor_mul(prod, u1s, u2)
                nc.scalar.activation(phiq[:, c * 512 : (c + 1) * 512], prod,
                                     AF.Tanh, scale=1.0 / SQRT_R)

            # ---- phi_k in [m, r] tiles and v_aug in [m, d|1] tiles
            phik = perb.tile([128, NT, R], F32, tag="phik")
            vau = perb.tile([128, NT, R + 1], F32, tag="vau")
            nc.vector.memset(vau[:, :, R], 1.0)  # ones column per tile
            for m in range(NT):
                kl = qk_sb[:, 1, m * 128 : (m + 1) * 128]
                u1 = psB.tile([128, 128], F32, tag="psB")
                u2 = psB.tile([128, 128], F32, tag="psB")
                nc.tensor.matmul(u1, lhsT=kl, rhs=g_s[:, 2, :], start=True, stop=True)
                nc.tensor.matmul(u2, lhsT=kl, rhs=g_s[:, 3, :], start=True, stop=True)
                u1ks = work.tile([128, 128], F32, tag="u1ks")
                nc.vector.tensor_copy(u1ks, u1)
                prodk = work.tile([128, 128], F32, tag="prodk")
                nc.vector.tensor_mul(prodk, u1ks, u2)
                nc.scalar.activation(phik[:, m, :], prodk, AF.Tanh,
                                     scale=1.0 / SQRT_R)
                vps = psB.tile([128, 128], F32, tag="psB")
                nc.tensor.matmul(
                    vps,
                    lhsT=xp_s[:, b, KS - 1 + m * 128 : KS - 1 + (m + 1) * 128],
                    rhs=vw_s, start=True, stop=True,
                )
                nc.scalar.copy(vau[:, m, 0:R], vps)

            # ---- M~' = [phi_k^T v | s_k]  ([r, R+1], accumulated over m tiles)
            mps = psM.tile([128, R + 1], F32, tag="psM")
            for m in range(NT):
                nc.tensor.matmul(mps, lhsT=phik[:, m, :], rhs=vau[:, m, :],
                                 start=(m == 0), stop=(m == NT - 1))
            m_sb = small.tile([128, R + 1], F32, tag="msb")
            nc.scalar.copy(m_sb, mps)

            # ---- tail per l-tile: [num | den] matmul, recip, transpose, proj
            for lt in range(NT):
                nd = psB.tile([128, R + 1], F32, tag="psB")
                nc.tensor.matmul(nd, lhsT=phiq[:, lt * 128 : (lt + 1) * 128],
                                 rhs=m_sb, start=True, stop=True)
                den = small.tile([128, 1], F32, tag="den")
                nc.vector.tensor_scalar_add(den, nd[:, R : R + 1], EPS)
                rec = small.tile([128, 1], F32, tag="rec")
                nc.vector.reciprocal(rec, den)
                num_sb = work.tile([128, 128], F32, tag="numsb")
                nc.scalar.copy(num_sb, nd[:, 0:R])
                ntp = psA.tile([128, 128], F32, tag="psA")
                nc.tensor.transpose(ntp, num_sb, ident)
                nt_sb = work.tile([128, 128], F32, tag="ntsb")
                nc.vector.tensor_copy(nt_sb, ntp)
                pps = psB.tile([128, 128], F32, tag="psB")
                nc.tensor.matmul(pps, lhsT=nt_sb, rhs=pw_s, start=True, stop=True)
                po = outs.tile([128, 128], F32, tag="po")
                nc.vector.tensor_scalar_mul(po, pps, rec)
                nc.sync.dma_start(out=out_d[b, lt * 128 : (lt + 1) * 128, :], in_=po)
    nc.compile()
    return nc


def _prep_in_maps(inputs):
    def f32(a):
        return np.ascontiguousarray(np.asarray(a), dtype=np.float32)

    x = f32(inputs["x"])                     # [B, D, L]
    q_w = f32(inputs["q_w"]).reshape(H, D, D, KS)
    k_w = f32(inputs["k_w"]).reshape(H, D, D, KS)
    v_w = f32(inputs["v_w"]).reshape(H, D, D)
    q_b = f32(inputs["q_b"]).reshape(H, D)
    k_b = f32(inputs["k_b"]).reshape(H, D)
    proj_w = f32(inputs["proj_w"])           # [D, H*D]
    gq = float(np.asarray(inputs["gamma_q"]).reshape(-1)[0])
    bq = float(np.asarray(inputs["beta_q"]).reshape(-1)[0])
    gk = float(np.asarray(inputs["gamma_k"]).reshape(-1)[0])
    bk = float(np.asarray(inputs["beta_k"]).reshape(-1)[0])

    xp = np.zeros((D, B, LP), np.float32)
    xp[:, :, KS - 1 :] = x.transpose(1, 0, 2)
    g_host = np.ascontiguousarray(
        np.stack([f32(inputs["g1_q"]), f32(inputs["g2_q"]),
                  f32(inputs["g1_k"]), f32(inputs["g2_k"])], axis=1)
    )  # [D, 4, R]

    in_maps = []
    for h in range(H):
        qkw = np.empty((D, 2, KS, D), np.float32)
        # [c, t, d] from [d, c, t]; fold gamma into weights
        qkw[:, 0] = (gq * q_w[h]).transpose(1, 2, 0)
        qkw[:, 1] = (gk * k_w[h]).transpose(1, 2, 0)
        qkb = np.empty((D, 2), np.float32)
        qkb[:, 0] = gq * q_b[h] + bq
        qkb[:, 1] = gk * k_b[h] + bk
        in_maps.append(dict(
            xp=xp,
            qkw=np.ascontiguousarray(qkw),
            qkb=np.ascontiguousarray(qkb),
            vw=np.ascontiguousarray(v_w[h].T),
            g=g_host,
            pw=np.ascontiguousarray(proj_w[:, h * D : (h + 1) * D].T),
        ))
    return in_maps


def kernel(**inputs):
    global _built_nc, last_results
    if _built_nc is None:
        _built_nc = _build()
    in_maps = _prep_in_maps(inputs)
    res = run_bass_kernel_spmd(_built_nc, in_maps, list(range(H)))
    last_results = res
    parts = np.stack([res.results[c]["outp"] for c in range(H)])
    out = parts.sum(axis=0, dtype=np.float32)
    out += np.asarray(inputs["proj_b"], np.float32)[None, None, :]
    return out.astype(np.float32)
